# revision 1
# baseline (speedup 1.0000x reference)
"""Trainium2 Bass kernel for nn_DecoderStory_71880572666639.

Architecture: 2-layer LSTM (H=512) scanned sequentially over the flattened
(B,T) sequence with ragged masking, followed by a vocab projection V=10000.

Strategy
--------
* Host: compact the scan to valid steps only (t < lengths[b]-1), build the
  layer-1 input x = [feats; emb] per valid step, pad to a multiple of 128.
* Device (identical NEFF on all 8 cores, data-parallel over vocab):
    phase B: X1 = W_ih1 @ x + b1 for all steps (batched GEMM, fp16 in, fp32
             psum, stored fp16 in SBUF as 16 per-gate-tile "planes").
    phase C: the sequential scan. Weights stationary in fp16 [128,128] tiles
             (LDWEIGHTS-streamed each step), h vectors moving [128,1].
             Gates land in PSUM as [128,16]; ACT adds X1[t] as per-partition
             bias and applies sigmoid/tanh; DVE does the c/h updates.
    phase D: logits = ys @ W_out_slice.T, each core owning 1250 vocab cols.
* Host: scatter valid rows into the [B,T,V] output, add b_out, prepend the
  fixed start vector.

Gate permutation: device gate index j = 128*m + p (column m in [0,16),
partition p) maps to torch-order gate g = (m//4)*512 + (m%4)*128 + p, so
columns 0-3 hold i, 4-7 f, 8-11 g~, 12-15 o, and hidden unit u = 128*c + p
lives at h[p, c] for c in [0,4).
"""

import os
import numpy as np

B, T, E, H, V = 64, 32, 256, 512, 10000
D1 = E + H            # 768
G = 4 * H             # 2048
P = 128
NCORES = 8
VSLICE = V // NCORES  # 1250
KC1 = D1 // P         # 6  K-chunks for the input projection
KH = H // P           # 4  K-chunks for one hidden vector
MG = G // P           # 16 gate tiles

# device gate permutation (device j -> torch gate index)
_m = np.arange(G) // P
_p = np.arange(G) % P
PERM = (_m // 4) * 512 + (_m % 4) * P + _p          # [2048]


def _round_up(x, mult):
    return ((x + mult - 1) // mult) * mult


# ---------------------------------------------------------------------------
# host-side packing
# ---------------------------------------------------------------------------

def _pack_stationary(Wp: np.ndarray, kchunks: int) -> np.ndarray:
    """Pack a permuted weight matrix Wp [G, K*?] into the SBUF stationary
    layout [128, (MG*kchunks)*128] fp16, block order b = m*kchunks + k,
    block(m, k)[kk, mm] = Wp[128*m + mm, 128*k + kk]."""
    ksz = Wp.shape[1]
    assert ksz == kchunks * P
    v = Wp.reshape(MG, P, kchunks, P)           # [m, mm, k, kk]
    v = v.transpose(3, 0, 2, 1)                 # [kk, m, k, mm]
    return np.ascontiguousarray(v.reshape(P, MG * kchunks * P)).astype(np.float16)


def _host_pack(story_feature, captions, lengths, W_story, b_story, embed,
               W_ih1, W_hh1, b1, W_ih2, W_hh2, b2, W_out, b_out):
    """All host-side preprocessing. Returns dict of device arrays + metadata."""
    f32 = np.float32
    feats = np.maximum(story_feature.astype(f32) @ W_story.T.astype(f32)
                       + b_story.astype(f32), 0.0)          # [B, H]

    lengths = lengths.astype(np.int64)
    valid_pairs = [(b, t) for b in range(B) for t in range(int(lengths[b]) - 1)]
    n_valid = len(valid_pairs)
    L = max(_round_up(n_valid, P), 2 * P)

    # x rows [feats; emb] in fp16, padded with zeros
    x = np.zeros((L, D1), np.float16)
    bs = np.array([p[0] for p in valid_pairs])
    ts = np.array([p[1] for p in valid_pairs])
    x[:n_valid, :H] = feats[bs].astype(np.float16)
    x[:n_valid, H:] = embed[captions[bs, ts]].astype(np.float16)

    # xts: x.T chunked [128, KC1 * L]
    xT = np.ascontiguousarray(x.T)                        # [768, L]
    xts = xT.reshape(KC1, P, L).transpose(1, 0, 2).reshape(P, KC1 * L)

    W1p = W_ih1[PERM].astype(f32)                         # [2048, 768]
    w1i = _pack_stationary(W1p, KC1)                      # [128, 96*128]
    b1s = b1[PERM].astype(np.float16).reshape(1, G)       # [1, 2048]

    whh1p = W_hh1[PERM].astype(f32)                       # [2048, 512]
    w1s = _pack_stationary(whh1p, KH)                     # [128, 64*128]

    W2 = np.concatenate([W_ih2, W_hh2], axis=1)[PERM].astype(f32)   # [2048, 1024]
    w2s = _pack_stationary(W2, 2 * KH)                    # [128, 128*128]

    b2t = np.ascontiguousarray(b2[PERM].reshape(MG, P).T).astype(f32)  # [128, 16]

    # per-core W_out slices: woutt[kk, c*VSLICE + v] = W_out[v0+v, 128c+kk]
    wouts = []
    for core in range(NCORES):
        Woc = W_out[core * VSLICE:(core + 1) * VSLICE].astype(f32)   # [1250, 512]
        wt = Woc.T.reshape(KH, P, VSLICE).transpose(1, 0, 2).reshape(P, KH * VSLICE)
        wouts.append(np.ascontiguousarray(wt).astype(np.float16))

    meta = dict(n_valid=n_valid, L=L, bs=bs, ts=ts)
    dev = dict(
        xts=np.ascontiguousarray(xts).astype(np.float16),
        w1i=w1i, b1s=b1s, w1s=w1s, w2s=w2s, b2t=b2t, wouts=wouts,
    )
    return dev, meta


# ---------------------------------------------------------------------------
# numpy mirror of the device program (layout validation)
# ---------------------------------------------------------------------------

def _numpy_device_sim(dev, L):
    """Simulates exactly what the device computes, using the packed layouts."""
    f32 = np.float32
    xts = dev["xts"].astype(f32).reshape(P, KC1, L)
    w1i = dev["w1i"].astype(f32)
    b1s = dev["b1s"].astype(f32)[0]
    w1s = dev["w1s"].astype(f32)
    w2s = dev["w2s"].astype(f32)
    b2t = dev["b2t"].astype(f32)

    # phase B: X1 planes [16][128, L] fp16
    X1 = np.zeros((MG, P, L), f32)
    for m in range(MG):
        acc = np.tile(b1s[P * m:P * (m + 1)][:, None], (1, L))
        for kc in range(KC1):
            blk = w1i[:, (m * KC1 + kc) * P:(m * KC1 + kc + 1) * P]   # [kk, mm]
            acc += blk.T @ xts[:, kc, :]
        X1[m] = acc
    X1 = X1.astype(np.float16).astype(f32)

    # reconstruct W1dev[j, i] (device gate j, flat h index i = 128k+kk)
    def unpack(ws, kchunks):
        W = np.zeros((G, kchunks * P), f32)
        for m in range(MG):
            for k in range(kchunks):
                blk = ws[:, (m * kchunks + k) * P:(m * kchunks + k + 1) * P]
                W[P * m:P * (m + 1), P * k:P * (k + 1)] = blk.T
        return W

    W1dev = unpack(w1s, KH)            # [2048, 512]
    W2dev = unpack(w2s, 2 * KH)        # [2048, 1024]
    b2flat = b2t.T.reshape(-1)         # [2048] device order

    def sig(v):
        return 1.0 / (1.0 + np.exp(-v))

    h1 = np.zeros(H, f32)   # flat device order i = 128c+p
    h2 = np.zeros(H, f32)
    c1 = np.zeros((P, KH), f32)
    c2 = np.zeros((P, KH), f32)
    YS = np.zeros((P, KH * L), np.float16)

    for t in range(L):
        g1f = W1dev @ h1
        g1 = g1f.reshape(MG, P).T + X1[:, :, t].T    # [P, MG]
        si, sf = sig(g1[:, 0:4]), sig(g1[:, 4:8])
        tg, so = np.tanh(g1[:, 8:12]), sig(g1[:, 12:16])
        c1 = sf * c1 + si * tg
        h1m = (so * np.tanh(c1)).astype(np.float16).astype(f32)   # [P, KH]
        h1 = h1m.T.reshape(-1)
        g2f = W2dev @ np.concatenate([h1, h2]) + b2flat
        g2 = g2f.reshape(MG, P).T
        si, sf = sig(g2[:, 0:4]), sig(g2[:, 4:8])
        tg, so = np.tanh(g2[:, 8:12]), sig(g2[:, 12:16])
        c2 = sf * c2 + si * tg
        h2m = (so * np.tanh(c2)).astype(np.float16).astype(f32)
        h2 = h2m.T.reshape(-1)
        YS[:, KH * t:KH * (t + 1)] = h2m.astype(np.float16)

    outs = []
    for core in range(NCORES):
        wt = dev["wouts"][core].astype(f32)               # [128, 4*1250]
        logits = np.zeros((L, VSLICE), f32)
        ysv = YS.astype(f32).reshape(P, L, KH)
        for c in range(KH):
            lhs = ysv[:, :, c]                            # [kk, steps]
            rhs = wt[:, c * VSLICE:(c + 1) * VSLICE]      # [kk, v]
            logits += lhs.T @ rhs
        outs.append(logits)
    return np.concatenate(outs, axis=1)                   # [L, V]


# ---------------------------------------------------------------------------
# device kernel build
# ---------------------------------------------------------------------------

_BUILD_CACHE = {}

SCAN_MODE = os.environ.get("SCAN_MODE", "for_i")   # "for_i" | "static"
UNROLL = int(os.environ.get("SCAN_UNROLL", "2"))


def _build(L):
    import concourse.bass as bass
    import concourse.tile as tile
    from concourse import bacc, mybir
    from concourse.bass import ds
    from contextlib import ExitStack

    F32 = mybir.dt.float32
    F16 = mybir.dt.float16
    AF = mybir.ActivationFunctionType

    nc = bacc.Bacc("TRN2", target_bir_lowering=False, debug=False,
                   num_devices=NCORES)

    xts_d = nc.dram_tensor("xts", [P, KC1 * L], F16, kind="ExternalInput").ap()
    w1i_d = nc.dram_tensor("w1i", [P, MG * KC1 * P], F16, kind="ExternalInput").ap()
    b1s_d = nc.dram_tensor("b1s", [1, G], F16, kind="ExternalInput").ap()
    w1s_d = nc.dram_tensor("w1s", [P, MG * KH * P], F16, kind="ExternalInput").ap()
    w2s_d = nc.dram_tensor("w2s", [P, MG * 2 * KH * P], F16, kind="ExternalInput").ap()
    b2t_d = nc.dram_tensor("b2t", [P, MG], F32, kind="ExternalInput").ap()
    wout_d = nc.dram_tensor("woutt", [P, KH * VSLICE], F16, kind="ExternalInput").ap()
    out_d = nc.dram_tensor("out", [L, VSLICE], F32, kind="ExternalOutput").ap()

    with tile.TileContext(nc) as tc:
        with ExitStack() as ctx:
            singles = ctx.enter_context(tc.tile_pool(name="singles", bufs=1))
            stage = ctx.enter_context(tc.tile_pool(name="stage", bufs=2))

            # --- load constants/weights ---
            w1s = singles.tile([P, MG * KH * P], F16)
            w2s = singles.tile([P, MG * 2 * KH * P], F16)
            w1i = singles.tile([P, MG * KC1 * P], F16)
            b1s = singles.tile([1, G], F16)
            b2t = singles.tile([P, MG], F32)
            woutt = singles.tile([P, KH * VSLICE], F16)
            ones = singles.tile([1, 512], F16)
            nc.sync.dma_start(out=w1s, in_=w1s_d)
            nc.sync.dma_start(out=w2s, in_=w2s_d)
            nc.sync.dma_start(out=w1i, in_=w1i_d)
            nc.sync.dma_start(out=b1s, in_=b1s_d)
            nc.sync.dma_start(out=b2t, in_=b2t_d)
            nc.sync.dma_start(out=woutt, in_=wout_d)
            nc.vector.memset(ones, 1.0)

            X1 = [singles.tile([P, L], F16, tag=f"x1_{m}", name=f"x1_{m}") for m in range(MG)]

            # --- phase B: input projection ---
            xts_v = xts_d.rearrange("p (k l) -> p k l", k=KC1)
            nts = [(o, min(512, L - o)) for o in range(0, L, 512)]
            with tc.tile_pool(name="xpool", bufs=2) as xpool, \
                 tc.tile_pool(name="pre_ps", bufs=4, space="PSUM") as pre_ps:
                for (off, nlen) in nts:
                    xsl = xpool.tile([P, KC1, 512], F16, tag="xsl")
                    nc.sync.dma_start(out=xsl[:, :, :nlen], in_=xts_v[:, :, off:off + nlen])
                    for m in range(MG):
                        ps = pre_ps.tile([P, 512], F32, tag="ps")
                        nc.tensor.matmul(ps[:, :nlen], b1s[0:1, P * m:P * (m + 1)],
                                         ones[0:1, :nlen], start=True, stop=False)
                        for kc in range(KC1):
                            blk = w1i[:, (m * KC1 + kc) * P:(m * KC1 + kc + 1) * P]
                            nc.tensor.matmul(ps[:, :nlen], blk, xsl[:, kc, :nlen],
                                             start=False, stop=(kc == KC1 - 1))
                        nc.scalar.copy(X1[m][:, off:off + nlen], ps[:, :nlen])

            # --- phase C: the scan ---
            h1r = [singles.tile([P, KH], F16, tag=f"h1_{u}", name=f"h1_{u}") for u in range(2)]
            h2r = [singles.tile([P, KH], F16, tag=f"h2_{u}", name=f"h2_{u}") for u in range(2)]
            c1r = [singles.tile([P, KH], F32, tag=f"c1_{u}", name=f"c1_{u}") for u in range(2)]
            c2r = [singles.tile([P, KH], F32, tag=f"c2_{u}", name=f"c2_{u}") for u in range(2)]
            YS = singles.tile([P, KH * L], F16)
            for u in range(2):
                nc.vector.memset(h1r[u], 0.0)
                nc.vector.memset(h2r[u], 0.0)
                nc.vector.memset(c1r[u], 0.0)
                nc.vector.memset(c2r[u], 0.0)

            def scan_step(scan_ps, u, t_expr, t_static=None):
                cur, prv = u % 2, 1 - (u % 2)
                g1 = scan_ps.tile([P, MG], F32, tag="g1")
                g2 = scan_ps.tile([P, MG], F32, tag="g2")
                sg1 = stage.tile([P, MG], F32, tag="sg1")
                sg2 = stage.tile([P, MG], F32, tag="sg2")
                tmp = stage.tile([P, 3 * KH], F32, tag="tmp")
                # layer-1 recurrent matvec
                for m in range(MG):
                    for k in range(KH):
                        blk = w1s[:, (m * KH + k) * P:(m * KH + k + 1) * P]
                        nc.tensor.matmul(g1[:, m:m + 1], blk, h1r[prv][:, k:k + 1],
                                         start=(k == 0), stop=(k == KH - 1))
                # gate nonlinearities (bias = X1[t])
                for m in range(MG):
                    func = AF.Tanh if 8 <= m < 12 else AF.Sigmoid
                    if t_static is not None:
                        bias = X1[m][:, t_static:t_static + 1]
                    else:
                        bias = X1[m][:, ds(t_expr, 1)]
                    nc.scalar.activation(sg1[:, m:m + 1], g1[:, m:m + 1], func,
                                         bias=bias, scale=1.0)
                # c1 = sf*c1 + si*tg ; h1 = so*tanh(c1)
                nc.vector.tensor_mul(tmp[:, 0:KH], sg1[:, 4:8], c1r[prv])
                nc.vector.tensor_mul(tmp[:, KH:2 * KH], sg1[:, 0:4], sg1[:, 8:12])
                nc.vector.tensor_add(c1r[cur], tmp[:, 0:KH], tmp[:, KH:2 * KH])
                nc.scalar.activation(tmp[:, 2 * KH:3 * KH], c1r[cur], AF.Tanh)
                nc.vector.tensor_mul(h1r[cur], sg1[:, 12:16], tmp[:, 2 * KH:3 * KH])
                # layer-2 matvec: h2 part first (no dep on h1r[cur])
                for m in range(MG):
                    for k in range(KH):
                        blk = w2s[:, (m * 2 * KH + KH + k) * P:(m * 2 * KH + KH + k + 1) * P]
                        nc.tensor.matmul(g2[:, m:m + 1], blk, h2r[prv][:, k:k + 1],
                                         start=(k == 0), stop=False)
                for m in range(MG):
                    for k in range(KH):
                        blk = w2s[:, (m * 2 * KH + k) * P:(m * 2 * KH + k + 1) * P]
                        nc.tensor.matmul(g2[:, m:m + 1], blk, h1r[cur][:, k:k + 1],
                                         start=False, stop=(k == KH - 1))
                for m in range(MG):
                    func = AF.Tanh if 8 <= m < 12 else AF.Sigmoid
                    nc.scalar.activation(sg2[:, m:m + 1], g2[:, m:m + 1], func,
                                         bias=b2t[:, m:m + 1], scale=1.0)
                nc.vector.tensor_mul(tmp[:, 0:KH], sg2[:, 4:8], c2r[prv])
                nc.vector.tensor_mul(tmp[:, KH:2 * KH], sg2[:, 0:4], sg2[:, 8:12])
                nc.vector.tensor_add(c2r[cur], tmp[:, 0:KH], tmp[:, KH:2 * KH])
                nc.scalar.activation(tmp[:, 2 * KH:3 * KH], c2r[cur], AF.Tanh)
                nc.vector.tensor_mul(h2r[cur], sg2[:, 12:16], tmp[:, 2 * KH:3 * KH])
                if t_static is not None:
                    nc.vector.tensor_copy(YS[:, KH * t_static:KH * (t_static + 1)],
                                          h2r[cur])
                else:
                    nc.vector.tensor_copy(YS[:, ds(t_expr * KH, KH)], h2r[cur])

            with tc.tile_pool(name="scan_ps", bufs=2, space="PSUM") as scan_ps:
                if SCAN_MODE == "static":
                    for t in range(L):
                        scan_step(scan_ps, t, None, t_static=t)
                else:
                    with tc.For_i(0, L // UNROLL, 1,
                                  hint_engines=(mybir.EngineType.PE,)) as i:
                        for u in range(UNROLL):
                            t_expr = nc.snap(i * UNROLL + u)
                            scan_step(scan_ps, u, t_expr)

            # --- phase D: vocab projection ---
            ys_v = YS.rearrange("p (l c) -> p c l", c=KH)
            vts = [(o, min(512, VSLICE - o)) for o in range(0, VSLICE, 512)]
            gemm_ps = ctx.enter_context(tc.tile_pool(name="gemm_ps", bufs=2, space="PSUM"))
            for sb in range(L // P):
                for (voff, vlen) in vts:
                    ps = gemm_ps.tile([P, 512], F32, tag="gps")
                    for c in range(KH):
                        nc.tensor.matmul(ps[:, :vlen],
                                         ys_v[:, c, P * sb:P * (sb + 1)],
                                         woutt[:, c * VSLICE + voff:c * VSLICE + voff + vlen],
                                         start=(c == 0), stop=(c == KH - 1))
                    st = stage.tile([P, 512], F32, tag="gst")
                    nc.scalar.copy(st[:, :vlen], ps[:, :vlen])
                    nc.sync.dma_start(out=out_d[P * sb:P * (sb + 1), voff:voff + vlen],
                                      in_=st[:, :vlen])

    nc.compile()
    return nc


# ---------------------------------------------------------------------------
# public entry point
# ---------------------------------------------------------------------------

LAST_RESULT = None


def kernel(story_feature, captions, lengths, W_story, b_story, embed,
           W_ih1, W_hh1, b1, W_ih2, W_hh2, b2, W_out, b_out):
    global LAST_RESULT
    from concourse import bass_utils

    dev, meta = _host_pack(story_feature, captions, lengths, W_story, b_story,
                           embed, W_ih1, W_hh1, b1, W_ih2, W_hh2, b2, W_out, b_out)
    L, n_valid = meta["L"], meta["n_valid"]

    key = (L, SCAN_MODE, UNROLL)
    if key not in _BUILD_CACHE:
        _BUILD_CACHE[key] = _build(L)
    nc = _BUILD_CACHE[key]

    in_maps = []
    for core in range(NCORES):
        in_maps.append(dict(
            xts=dev["xts"], w1i=dev["w1i"], b1s=dev["b1s"], w1s=dev["w1s"],
            w2s=dev["w2s"], b2t=dev["b2t"], woutt=dev["wouts"][core],
        ))
    trace = os.environ.get("BASS_TRACE", "0") == "1"
    res = bass_utils.run_bass_kernel_spmd(nc, in_maps, core_ids=list(range(NCORES)),
                                          trace=trace)
    LAST_RESULT = res

    logits = np.concatenate([res.results[c]["out"] for c in range(NCORES)],
                            axis=1)            # [L, V]
    return _host_post(logits, meta, b_out)


def _host_post(logits, meta, b_out):
    n_valid, bs, ts = meta["n_valid"], meta["bs"], meta["ts"]
    out = np.zeros((B, T, V), np.float32)
    out[:, 0, 1] = 10000.0
    rows = logits[:n_valid] + b_out.astype(np.float32)[None, :]
    # valid step (b, t) writes output position (b, t+1)
    out[bs, ts + 1] = rows
    return out


def kernel_numpy_ref(story_feature, captions, lengths, W_story, b_story, embed,
                     W_ih1, W_hh1, b1, W_ih2, W_hh2, b2, W_out, b_out):
    """Pure-numpy end-to-end mirror of the device pipeline (layout check)."""
    dev, meta = _host_pack(story_feature, captions, lengths, W_story, b_story,
                           embed, W_ih1, W_hh1, b1, W_ih2, W_hh2, b2, W_out, b_out)
    logits = _numpy_device_sim(dev, meta["L"])
    return _host_post(logits, meta, b_out)

